# revision 7
# baseline (speedup 1.0000x reference)
"""AttentionBlock (GroupNorm + 1x1-conv QKV + MHSA + 1x1-conv proj + residual)
for Trainium2, data-parallel over batch across 8 NeuronCores.

Per-core layout (batch element b):
  x          [256, 1024]   channels on partitions (2 tiles of 128), pixels free
  groupnorm  stats via DVE free-dim reduces + tiny PE matmuls with 0/1 group
             matrices (partition-dim reduction / expansion)
  qk = W_qk @ xn   [512, 1024]  (bias folded in via an appended ones-row
                                 contraction term)
  vT = xn^T @ W_v  [1024, 256]  (v transposed "for free" by swapping matmul
                                 operands; bias folded the same way)
  scores_T[h] = K_h^T-contraction:  S_T[j,i] = sum_d K[d,j] Q[d,i]
             -> exp on ScalarE (no max-subtraction needed: |s|*scale < ~10)
  numerator[h] = [V_h^T | ones]^T @ E_T  -> rows 0:32 = numerator,
             rows 32:64 = softmax denominator replicated across partitions
  normalize  via DVE reciprocal + multiply
  proj + bias + residual, DMA out.
"""

import numpy as np

import concourse.bass as bass
import concourse.tile as tile
from concourse import mybir
from concourse.bass_utils import run_bass_kernel_spmd

# ---------------------------------------------------------------------------
# Workaround for the bundled walrus: an unstructured instruction may carry at
# most ONE sync-wait, but TileContext's tail drain attaches one wait per live
# processor clock.  Split them across multiple drain instructions.
from bass_rust import ScopedClock, VectorClock


def _patched_drain_and_barrier(self, tick_clock, wait_clock):
    nc = self.nc
    vals_all = list(tick_clock.global_clock)
    for p, v in enumerate(vals_all):
        if not v:
            continue
        vals = [0] * len(vals_all)
        vals[p] = v
        d = nc.sync.drain()
        wait_clock.add_sem_waits(d.ins, ScopedClock({None: VectorClock(vals)}))
    nc.all_engine_barrier()
    assert self.sems is not None
    popped = nc._tile_sem_poison_stack.pop()
    assert popped is self._sem_poison
    nc.clear_and_free_semaphores(list(self.sems.allocated().values()))
    nc.all_engine_barrier()


tile.TileContext._drain_and_barrier = _patched_drain_and_barrier

_COMPUTE_ENGINES = (
    mybir.EngineType.PE,
    mybir.EngineType.Activation,
    mybir.EngineType.DVE,
    mybir.EngineType.Pool,
    mybir.EngineType.SP,
)


def _split_multi_waits(nc):
    """Hoist excess sync-waits onto same-engine NoOps so every instruction
    carries at most one wait (this walrus build rejects more)."""
    uid = 0
    for f in nc.m.functions:
        for bb in f.blocks:
            new = []
            for inst in bb.instructions:
                si = inst.sync_info
                if (
                    si is not None
                    and si.on_wait
                    and len(si.on_wait) > 1
                    and inst.engine in _COMPUTE_ENGINES
                ):
                    for w in si.on_wait[:-1]:
                        nop = mybir.InstNoOp(name=f"waitnop_{uid}", ins=[], outs=[])
                        uid += 1
                        nop.engine = inst.engine
                        nop.sync_info = mybir.SyncInfo(on_wait=[w], on_update=[])
                        new.append(nop)
                    si.on_wait = [si.on_wait[-1]]
                new.append(inst)
            bb.instructions[:] = new
# ---------------------------------------------------------------------------

F32 = mybir.dt.float32
AF = mybir.ActivationFunctionType
ALU = mybir.AluOpType
AX = mybir.AxisListType

B = 8          # batch (== number of cores)
C = 256        # channels
N = 1024       # pixels (32*32)
NH = 8         # heads
HD = 32        # head dim
GROUPS = 32
GSIZE = C // GROUPS          # channels per group (8)
EPS = 1e-5
SCALE = float(HD) ** -0.5
GELEMS = float(GSIZE * N)    # elements per group


def build_nc(repeat: int = 1) -> bass.Bass:
    nc = bass.Bass()

    x_d = nc.dram_tensor("x", [C, N], F32, kind="ExternalInput")
    wqkv_d = nc.dram_tensor("wqkv_aug", [C + 1, 3 * C], F32, kind="ExternalInput")
    wproj_d = nc.dram_tensor("wproj_aug", [C + 1, C], F32, kind="ExternalInput")
    gamma_d = nc.dram_tensor("gamma2", [128, 2], F32, kind="ExternalInput")
    beta_d = nc.dram_tensor("beta2", [128, 2], F32, kind="ExternalInput")
    g_d = nc.dram_tensor("gmat", [2, 128, GROUPS], F32, kind="ExternalInput")
    gt_d = nc.dram_tensor("gtmat", [2, GROUPS, 128], F32, kind="ExternalInput")
    y_d = nc.dram_tensor("y", [C, N], F32, kind="ExternalOutput")

    with tile.TileContext(nc) as tc:
        with (
            tc.tile_pool(name="persist", bufs=1) as pp,
            tc.tile_pool(name="work", bufs=2) as wp,
            tc.tile_pool(name="epool", bufs=3) as ep,
        ):
            # ---- constant loads (once) ----
            wqkvA = pp.tile([128, 3 * C], F32, tag="wqkvA")
            wqkvB = pp.tile([128, 3 * C], F32, tag="wqkvB")
            wqkvC = pp.tile([1, 3 * C], F32, tag="wqkvC")
            wprojA = pp.tile([128, C], F32, tag="wprojA")
            wprojB = pp.tile([128, C], F32, tag="wprojB")
            wprojC = pp.tile([1, C], F32, tag="wprojC")
            gamma2 = pp.tile([128, 2], F32, tag="gamma2")
            beta2 = pp.tile([128, 2], F32, tag="beta2")
            g0 = pp.tile([128, GROUPS], F32, tag="g0")
            g1 = pp.tile([128, GROUPS], F32, tag="g1")
            gt0 = pp.tile([GROUPS, 128], F32, tag="gt0")
            gt1 = pp.tile([GROUPS, 128], F32, tag="gt1")
            ones = pp.tile([1, N], F32, tag="ones")

            nc.sync.dma_start(wqkvA, wqkv_d[0:128, :])
            nc.sync.dma_start(wqkvB, wqkv_d[128:256, :])
            nc.sync.dma_start(wqkvC, wqkv_d[256:257, :])
            nc.sync.dma_start(wprojA, wproj_d[0:128, :])
            nc.sync.dma_start(wprojB, wproj_d[128:256, :])
            nc.sync.dma_start(wprojC, wproj_d[256:257, :])
            nc.sync.dma_start(gamma2, gamma_d[:, :])
            nc.sync.dma_start(beta2, beta_d[:, :])
            nc.sync.dma_start(g0, g_d[0])
            nc.sync.dma_start(g1, g_d[1])
            nc.sync.dma_start(gt0, gt_d[0])
            nc.sync.dma_start(gt1, gt_d[1])
            nc.vector.memset(ones, 1.0)

            for rep in range(repeat):
                r = ""  # tiles reused across repeats; slot deps serialize reps
                xA = pp.tile([128, N], F32, tag=r + "xA")
                xB = pp.tile([128, N], F32, tag=r + "xB")
                nc.sync.dma_start(xA, x_d[0:128, :])
                nc.sync.dma_start(xB, x_d[128:256, :])

                xn = [pp.tile([128, N], F32, tag=r + f"xn{t}", name=r + f"xn{t}") for t in range(2)]

                # ================= GroupNorm =================
                stats = []
                for t, xt in enumerate((xA, xB)):
                    sq = wp.tile([128, N], F32, tag="sq")
                    nc.vector.tensor_tensor(out=sq, in0=xt, in1=xt, op=ALU.mult)
                    st = wp.tile([128, 2], F32, tag=f"st{t}")
                    nc.vector.tensor_reduce(
                        out=st[:, 0:1], in_=xt, axis=AX.X, op=ALU.add
                    )
                    nc.vector.tensor_reduce(
                        out=st[:, 1:2], in_=sq, axis=AX.X, op=ALU.add
                    )
                    stats.append(st)

                with tc.tile_pool(name="ps_small", bufs=1, space="PSUM") as pss:
                    gst = pss.tile([GROUPS, 2], F32, tag="gst")
                    nc.tensor.matmul(gst, g0, stats[0], start=True, stop=False)
                    nc.tensor.matmul(gst, g1, stats[1], start=False, stop=True)

                    mv = wp.tile([GROUPS, 2], F32, tag="mv")
                    # mv = [mean, E[x^2]]
                    nc.vector.tensor_scalar_mul(mv, gst, 1.0 / GELEMS)
                    m2 = wp.tile([GROUPS, 1], F32, tag="m2")
                    nc.vector.tensor_tensor(
                        out=m2, in0=mv[:, 0:1], in1=mv[:, 0:1], op=ALU.mult
                    )
                    var = wp.tile([GROUPS, 1], F32, tag="var")
                    nc.vector.tensor_tensor(
                        out=var, in0=mv[:, 1:2], in1=m2, op=ALU.subtract
                    )
                    vare = wp.tile([GROUPS, 1], F32, tag="vare")
                    nc.vector.tensor_scalar_add(vare, var, EPS)
                    # rstd = (var+eps)^-0.5 = exp(-0.5*ln(var+eps)); keeps the
                    # whole kernel on the exp/ln ACT table set (no switch) and
                    # avoids the low-precision Sqrt table.
                    lnv = wp.tile([GROUPS, 1], F32, tag="lnv")
                    nc.scalar.activation(lnv, vare, AF.Ln)
                    nc.scalar.activation(mv[:, 1:2], lnv, AF.Exp, scale=-0.5)

                    for t, xt in enumerate((xA, xB)):
                        cst = pss.tile([128, 2], F32, tag=f"cst{t}")
                        nc.tensor.matmul(
                            cst, (gt0, gt1)[t], mv, start=True, stop=True
                        )
                        a_t = wp.tile([128, 1], F32, tag=f"a{t}")
                        nc.vector.tensor_tensor(
                            out=a_t, in0=cst[:, 1:2], in1=gamma2[:, t : t + 1],
                            op=ALU.mult,
                        )
                        mb = wp.tile([128, 1], F32, tag=f"mb{t}")
                        nc.vector.tensor_tensor(
                            out=mb, in0=cst[:, 0:1], in1=a_t, op=ALU.mult
                        )
                        b2 = wp.tile([128, 1], F32, tag=f"b2{t}")
                        nc.vector.tensor_tensor(
                            out=b2, in0=beta2[:, t : t + 1], in1=mb,
                            op=ALU.subtract,
                        )
                        nc.vector.tensor_scalar(
                            out=xn[t], in0=xt, scalar1=a_t, scalar2=b2,
                            op0=ALU.mult, op1=ALU.add,
                        )

                # ================= QKV =================
                qk = [pp.tile([128, N], F32, tag=r + f"qk{m}", name=r + f"qk{m}") for m in range(4)]
                vt = [
                    pp.tile([128, NH, 2 * HD], F32, tag=r + f"vt{j}", name=r + f"vt{j}")
                    for j in range(8)
                ]

                with tc.tile_pool(name="ps_qkv", bufs=2, space="PSUM") as psq:
                    for m in range(4):
                        pt = psq.tile([128, N], F32, tag="qkps")
                        for cch in range(2):
                            o = pt[:, cch * 512 : (cch + 1) * 512]
                            csl = slice(cch * 512, (cch + 1) * 512)
                            nc.tensor.matmul(
                                o, wqkvA[:, m * 128 : (m + 1) * 128],
                                xn[0][:, csl], start=True, stop=False,
                            )
                            nc.tensor.matmul(
                                o, wqkvB[:, m * 128 : (m + 1) * 128],
                                xn[1][:, csl], start=False, stop=False,
                            )
                            nc.tensor.matmul(
                                o, wqkvC[:, m * 128 : (m + 1) * 128],
                                ones[:, csl], start=False, stop=True,
                            )
                        nc.vector.tensor_copy(qk[m], pt)

                    for j in range(8):
                        jsl = slice(j * 128, (j + 1) * 128)
                        pt = psq.tile([128, C], F32, tag="vtps")
                        nc.tensor.matmul(
                            pt, xn[0][:, jsl], wqkvA[:, 512:768],
                            start=True, stop=False,
                        )
                        nc.tensor.matmul(
                            pt, xn[1][:, jsl], wqkvB[:, 512:768],
                            start=False, stop=False,
                        )
                        nc.tensor.matmul(
                            pt, ones[:, jsl], wqkvC[:, 512:768],
                            start=False, stop=True,
                        )
                        # scatter v^T into per-head [vT_h | ones] weight blocks
                        nc.vector.tensor_copy(
                            vt[j][:, :, 0:HD],
                            pt.rearrange("p (h d) -> p h d", h=NH),
                        )
                        nc.vector.memset(vt[j][:, :, HD : 2 * HD], 1.0)

                # ================= Attention =================
                ao = [pp.tile([128, N], F32, tag=r + f"ao{t}", name=r + f"ao{t}") for t in range(2)]
                with (
                    tc.tile_pool(name="ps_scr", bufs=2, space="PSUM") as pscr_pool,
                    tc.tile_pool(name="ps_num", bufs=2, space="PSUM") as pnum_pool,
                ):
                    for h in range(NH):
                        hp = 32 * (h % 4)
                        qsl = qk[h // 4][hp : hp + 32, :]
                        ksl = qk[2 + h // 4][hp : hp + 32, :]
                        pn = pnum_pool.tile([2 * HD, N], F32, tag="pn")
                        for j in range(8):
                            jsl = slice(j * 128, (j + 1) * 128)
                            ps = pscr_pool.tile([128, N], F32, tag="scores")
                            for cch in range(2):
                                csl = slice(cch * 512, (cch + 1) * 512)
                                nc.tensor.matmul(
                                    ps[:, csl], ksl[:, jsl], qsl[:, csl],
                                    start=True, stop=True,
                                    tile_position=(hp, 0),
                                )
                            e_t = ep.tile([128, N], F32, tag="E")
                            nc.scalar.activation(e_t, ps, AF.Exp, scale=SCALE)
                            for cch in range(2):
                                csl = slice(cch * 512, (cch + 1) * 512)
                                nc.tensor.matmul(
                                    pn[:, csl], vt[j][:, h, :], e_t[:, csl],
                                    start=(j == 0), stop=(j == 7),
                                )
                        # normalize: rows 0:32 numerator, rows 32:64 denominator
                        rsb = wp.tile([2 * HD, N], F32, tag="rsb")
                        nc.vector.reciprocal(rsb[HD:, :], pn[HD:, :])
                        nc.vector.tensor_tensor(
                            out=ao[h // 4][hp : hp + 32, :],
                            in0=pn[0:HD, :], in1=rsb[HD:, :], op=ALU.mult,
                        )

                # ================= Proj + residual =================
                with tc.tile_pool(name="ps_proj", bufs=2, space="PSUM") as psp:
                    for m in range(2):
                        msl = slice(m * 128, (m + 1) * 128)
                        pt = psp.tile([128, N], F32, tag="projps")
                        for cch in range(2):
                            csl = slice(cch * 512, (cch + 1) * 512)
                            o = pt[:, csl]
                            nc.tensor.matmul(
                                o, wprojA[:, msl], ao[0][:, csl],
                                start=True, stop=False,
                            )
                            nc.tensor.matmul(
                                o, wprojB[:, msl], ao[1][:, csl],
                                start=False, stop=False,
                            )
                            nc.tensor.matmul(
                                o, wprojC[:, msl], ones[:, csl],
                                start=False, stop=True,
                            )
                        yt = wp.tile([128, N], F32, tag=f"y{m}")
                        nc.vector.tensor_tensor(
                            out=yt, in0=pt, in1=(xA, xB)[m], op=ALU.add
                        )
                        nc.sync.dma_start(y_d[msl, :], yt)

    _split_multi_waits(nc)
    return nc


def make_in_maps(x, gamma, beta, w_qkv, b_qkv, w_proj, b_proj):
    x = np.ascontiguousarray(x, dtype=np.float32)
    wqkv_aug = np.concatenate(
        [np.ascontiguousarray(w_qkv.T), b_qkv[None, :]], axis=0
    ).astype(np.float32)
    wproj_aug = np.concatenate(
        [np.ascontiguousarray(w_proj.T), b_proj[None, :]], axis=0
    ).astype(np.float32)
    gamma2 = np.ascontiguousarray(gamma.reshape(2, 128).T).astype(np.float32)
    beta2 = np.ascontiguousarray(beta.reshape(2, 128).T).astype(np.float32)
    g = np.zeros((2, 128, GROUPS), dtype=np.float32)
    for t in range(2):
        for p in range(128):
            g[t, p, (t * 128 + p) // GSIZE] = 1.0
    gt = np.ascontiguousarray(np.transpose(g, (0, 2, 1)))

    shared = {
        "wqkv_aug": wqkv_aug,
        "wproj_aug": wproj_aug,
        "gamma2": gamma2,
        "beta2": beta2,
        "gmat": g,
        "gtmat": gt,
    }
    return [
        {"x": np.ascontiguousarray(x[b].reshape(C, N)), **shared} for b in range(B)
    ]


def kernel(x, gamma, beta, w_qkv, b_qkv, w_proj, b_proj):
    nc = build_nc()
    in_maps = make_in_maps(x, gamma, beta, w_qkv, b_qkv, w_proj, b_proj)
    res = run_bass_kernel_spmd(nc, in_maps, list(range(B)))
    out = np.stack([res.results[b]["y"].reshape(C, 32, 32) for b in range(B)])
    return out.astype(np.float32)
